# revision 2
# baseline (speedup 1.0000x reference)
"""CoreHybridBlock Trainium2 kernel v2: fully transposed layout.

Per-core program (one batch element per core), everything in
[feature(part), token(free)] layout; host pre/post-transposes x,v.

Key ideas vs v1:
 - C=512 chunks (PSUM bank = exactly [128,512] f32 -> 2x PSUM efficiency)
 - no PE transposes: rmsnorm done in transposed space (partition-reduce
   via ones-matmul, per-token scale broadcast via gpsimd partition_broadcast)
 - all fat matmuls bf16; FFN w1/w3/w2 in fp8e4m3 + DoubleRow (2x PE)
 - ssm_out folded into out_proj on host (wcat)
 - dt chain: Softplus activation directly (pre-clip proved redundant)
 - elementwise offloaded across DVE / Scalar(Act) / GpSimd(Pool)
"""

import ml_dtypes
import numpy as np
import bass_rust
import concourse.bass as bass
import concourse.tile as tile
from concourse import mybir
from concourse.bass_utils import run_bass_kernel_spmd

F32 = mybir.dt.float32
F32R = mybir.dt.float32r
BF16 = mybir.dt.bfloat16
FP8 = mybir.dt.float8e4
AF = mybir.ActivationFunctionType
OP = mybir.AluOpType
DR = mybir.MatmulPerfMode.DoubleRow

D_MODEL, D_CONV, D_MAMBA = 512, 256, 256
DSTATE, N_HEADS, KCONV, FFN = 64, 4, 3, 2048
EPS = 1e-6

# fp8 scales (powers of two; folded on host / in activation scales)
SA = 16.0     # x2n activation scale
S1 = 512.0    # w1 weight scale
SH = 64.0     # h activation scale
S3 = SH / SA  # w3 weight scale: makes ps_b = b_true * SH so h = silu(a) * ps_b
S2 = 512.0    # w2 weight scale


# ---------------------------------------------------------------- wait split
def split_waits(nc, max_w=1):
    """walrus rejects >1 sync wait per instruction on some types; hoist
    excess waits onto same-engine NoOps."""
    cnt = 0
    for f in nc.m.functions:
        for bb in f.blocks:
            new_list = []
            changed = False
            for inst in bb.instructions:
                si = inst.sync_info
                waits = list(si.on_wait) if si is not None and si.on_wait else []
                if len(waits) > max_w:
                    changed = True
                    extra = waits[max_w:]
                    si.on_wait = waits[:max_w]
                    for j in range(0, len(extra), max_w):
                        cnt += 1
                        nop = bass_rust.InstNoOp(
                            name=f"I-waitsplit-{cnt}", ins=[], outs=[]
                        )
                        nop.engine = inst.engine
                        nop.sync_info = bass_rust.SyncInfo(
                            on_wait=extra[j : j + max_w], on_update=[]
                        )
                        new_list.append(nop)
                new_list.append(inst)
            if changed:
                bb.instructions = new_list
    return cnt


# ---------------------------------------------------------------- program
def build_program(L, C, beta, split=True):
    NCH = L // C
    nc = bass.Bass()

    # ---- dram I/O (transposed: [feature, token])
    x_d = nc.dram_tensor("x", [D_MODEL, L], F32, kind="ExternalInput")
    v_d = nc.dram_tensor("v", [D_MODEL, L], F32, kind="ExternalInput")
    wconv_d = nc.dram_tensor("w_conv", [D_MODEL, 2 * D_CONV], BF16, kind="ExternalInput")
    wxp_d = nc.dram_tensor("w_xproj", [D_MODEL, D_MAMBA], BF16, kind="ExternalInput")
    wdt_d = nc.dram_tensor("w_dt", [D_MODEL, D_MAMBA], BF16, kind="ExternalInput")
    wbc_d = nc.dram_tensor("w_bc", [D_MODEL, 2 * DSTATE], BF16, kind="ExternalInput")
    wcat_d = nc.dram_tensor("w_cat", [D_MODEL, D_MODEL], BF16, kind="ExternalInput")
    w1_d = nc.dram_tensor("w1dr", [2 * 128, 2, FFN], FP8, kind="ExternalInput")
    w3_d = nc.dram_tensor("w3dr", [2 * 128, 2, FFN], FP8, kind="ExternalInput")
    w2_d = nc.dram_tensor("w2dr", [8 * 128, 2, D_MODEL], FP8, kind="ExternalInput")
    avec_d = nc.dram_tensor("a_vec", [D_MAMBA, 1], F32, kind="ExternalInput")
    dtb_d = nc.dram_tensor("dtb_vec", [D_MAMBA, 1], F32, kind="ExternalInput")
    dvec_d = nc.dram_tensor("d_vec", [D_MAMBA, 1], F32, kind="ExternalInput")
    convb_d = nc.dram_tensor("convb_vec", [D_CONV, 1], F32, kind="ExternalInput")
    convw_d = nc.dram_tensor("convw", [D_CONV, KCONV], F32, kind="ExternalInput")
    mask2_d = nc.dram_tensor("mask2", [128, 2], BF16, kind="ExternalInput")
    selmix_d = nc.dram_tensor("selmix", [2, 128], BF16, kind="ExternalInput")
    dup64_d = nc.dram_tensor("dup64", [64, 128], BF16, kind="ExternalInput")
    ones_d = nc.dram_tensor("ones128", [128, 1], BF16, kind="ExternalInput")
    ones1_d = nc.dram_tensor("ones1", [1, 128], BF16, kind="ExternalInput")

    xo_d = nc.dram_tensor("x_out", [D_MODEL, L], F32, kind="ExternalOutput")
    vo_d = nc.dram_tensor("v_out", [D_MODEL, L], F32, kind="ExternalOutput")

    from contextlib import ExitStack

    with tile.TileContext(nc) as tc, ExitStack() as es:
        ec = es.enter_context
        cp = ec(tc.tile_pool(name="consts", bufs=1))
        sp = ec(tc.tile_pool(name="state", bufs=1))
        pin = ec(tc.tile_pool(name="pin", bufs=8))        # x tiles (f32)
        pvin = ec(tc.tile_pool(name="pvin", bufs=5))      # v tiles (f32)
        psq = ec(tc.tile_pool(name="psq", bufs=3))        # sq tiles (bf16)
        pxn = ec(tc.tile_pool(name="pxn", bufs=8))        # xn tiles (bf16)
        prs = ec(tc.tile_pool(name="prs", bufs=2))        # r_sb broadcast (f32)
        prt = ec(tc.tile_pool(name="prt", bufs=2))        # r [1,C] tiles
        pconv = ec(tc.tile_pool(name="pconv", bufs=2))
        pssm = ec(tc.tile_pool(name="pssm", bufs=2))
        pbc = ec(tc.tile_pool(name="pbc", bufs=2))
        px2 = ec(tc.tile_pool(name="px2", bufs=6))        # x2 tiles f32
        pvn = ec(tc.tile_pool(name="pvn", bufs=4))
        px2n = ec(tc.tile_pool(name="px2n", bufs=2))      # fp8 DR act tiles
        phdr = ec(tc.tile_pool(name="phdr", bufs=2))      # fp8 DR h tiles
        psil = ec(tc.tile_pool(name="psil", bufs=3))
        pxo = ec(tc.tile_pool(name="pxo", bufs=4))
        psA = ec(tc.tile_pool(name="psA", bufs=3, space="PSUM"))
        psB = ec(tc.tile_pool(name="psB", bufs=2, space="PSUM"))
        psF = ec(tc.tile_pool(name="psF", bufs=1, space="PSUM"))
        if True:
            def mm(out, lhsT, rhs, start, stop, pm=None):
                nc.tensor.matmul(
                    out=out, lhsT=lhsT, rhs=rhs, start=start, stop=stop,
                    perf_mode=pm,
                )

            # ---------------- constants / weights resident in SBUF
            def load_const(name, dram_ap, shape, dt):
                t = cp.tile(shape, dt, name=name, tag=name)
                nc.sync.dma_start(out=t, in_=dram_ap)
                return t

            wconv_sb = [
                load_const(f"wconv{k}", wconv_d[k * 128:(k + 1) * 128, :], [128, 2 * D_CONV], BF16)
                for k in range(4)
            ]
            wxp_sb = [
                load_const(f"wxp{k}", wxp_d[k * 128:(k + 1) * 128, :], [128, D_MAMBA], BF16)
                for k in range(4)
            ]
            wdt_sb = [
                load_const(f"wdt{k}", wdt_d[k * 128:(k + 1) * 128, :], [128, D_MAMBA], BF16)
                for k in range(4)
            ]
            wbc_sb = [
                load_const(f"wbc{k}", wbc_d[k * 128:(k + 1) * 128, :], [128, 2 * DSTATE], BF16)
                for k in range(4)
            ]
            wcat_sb = [
                load_const(f"wcat{k}", wcat_d[k * 128:(k + 1) * 128, :], [128, D_MODEL], BF16)
                for k in range(4)
            ]
            avec = [load_const(f"avec{m}", avec_d[m * 128:(m + 1) * 128, :], [128, 1], F32) for m in range(2)]
            dtb = [load_const(f"dtb{m}", dtb_d[m * 128:(m + 1) * 128, :], [128, 1], F32) for m in range(2)]
            dvec = [load_const(f"dvec{m}", dvec_d[m * 128:(m + 1) * 128, :], [128, 1], F32) for m in range(2)]
            convb = [load_const(f"convb{m}", convb_d[m * 128:(m + 1) * 128, :], [128, 1], F32) for m in range(2)]
            convw = [load_const(f"convw{m}", convw_d[m * 128:(m + 1) * 128, :], [128, KCONV], F32) for m in range(2)]
            mask2 = load_const("mask2", mask2_d[:, :], [128, 2], BF16)
            selmix = load_const("selmix", selmix_d[:, :], [2, 128], BF16)
            dup64 = load_const("dup64", dup64_d[:, :], [64, 128], BF16)
            ones128 = load_const("ones128", ones_d[:, :], [128, 1], BF16)
            ones1 = load_const("ones1", ones1_d[:, :], [1, 128], BF16)
            # FFN weights (fp8 DR layout) — loaded last so early chunks
            # don't wait on them
            w1_sb = [
                load_const(f"w1dr{j}", w1_d[j * 128:(j + 1) * 128, :, :], [128, 2, FFN], FP8)
                for j in range(2)
            ]
            w3_sb = [
                load_const(f"w3dr{j}", w3_d[j * 128:(j + 1) * 128, :, :], [128, 2, FFN], FP8)
                for j in range(2)
            ]
            w2_sb = [
                load_const(f"w2dr{j}", w2_d[j * 128:(j + 1) * 128, :, :], [128, 2, D_MODEL], FP8)
                for j in range(8)
            ]

            eps_sb = cp.tile([128, 1], F32, name="eps_sb", tag="eps_sb")
            nc.vector.memset(eps_sb, EPS)
            eps2_sb = cp.tile([128, 1], F32, name="eps2_sb", tag="eps2_sb")
            nc.vector.memset(eps2_sb, EPS / (SA * SA))
            one_sb = cp.tile([128, 1], F32, name="one_sb", tag="one_sb")
            nc.vector.memset(one_sb, 1.0)

            # ---------------- persistent cross-chunk state
            h_st = [sp.tile([128, 1], F32, name=f"hst{m}", tag=f"hst{m}") for m in range(2)]
            u_halo = [sp.tile([128, 2], F32, name=f"uhalo{m}", tag=f"uhalo{m}") for m in range(2)]
            for m in range(2):
                nc.vector.memset(h_st[m], 0.0)
                nc.vector.memset(u_halo[m], 0.0)

            # ---------------- helpers for the software-pipelined loop
            def emit_loads(c):
                col0 = c * C
                xk, vk = [], []
                for k in range(4):
                    xt = pin.tile([128, C], F32, name="xk", tag="xk")
                    nc.sync.dma_start(out=xt, in_=x_d[k * 128:(k + 1) * 128, col0:col0 + C])
                    xk.append(xt)
                for k in range(4):
                    vt = pvin.tile([128, C], F32, name="vk", tag="vk")
                    nc.sync.dma_start(out=vt, in_=v_d[k * 128:(k + 1) * 128, col0:col0 + C])
                    vk.append(vt)
                return xk, vk

            # 1/sqrt(z) = exp(-0.5*ln(z)): keeps the scalar engine inside the
            # natural_log_exp act table (no table switch, no DVE reciprocal)
            def emit_ss_ln_exp(src_tiles, scale, bias_t, tag, sq_pool_eng):
                ps_ss = psA.tile([128, C], F32, name="psA", tag="psA")
                for k in range(4):
                    sq = psq.tile([128, C], BF16, name="sq", tag="sq")
                    if sq_pool_eng:
                        nc.gpsimd.tensor_mul(out=sq, in0=src_tiles[k], in1=src_tiles[k])
                    else:
                        nc.scalar.activation(out=sq, in_=src_tiles[k], func=AF.Square)
                    mm(out=ps_ss[0:1, :], lhsT=ones128, rhs=sq,
                       start=(k == 0), stop=(k == 3))
                rt = prt.tile([1, C], F32, name=tag, tag=tag)
                nc.scalar.activation(
                    out=rt, in_=ps_ss[0:1, :], func=AF.Ln,
                    scale=scale, bias=bias_t[0:1],
                )
                rb = prt.tile([1, C], BF16, name=tag + "r", tag=tag + "r")
                nc.scalar.activation(out=rb, in_=rt, func=AF.Exp, scale=-0.5)
                return rb

            def emit_rmm(rb):
                ps_r = psA.tile([128, C], F32, name="psA", tag="psA")
                mm(out=ps_r, lhsT=ones1, rhs=rb, start=True, stop=True)
                return ps_r

            def emit_xn(xk, ps_r):
                xn = []
                for k in range(4):
                    t = pxn.tile([128, C], BF16, name="xn", tag="xn")
                    nc.vector.tensor_mul(out=t, in0=xk[k], in1=ps_r)
                    xn.append(t)
                return xn

            # ---------------- main chunk loop (next chunk's loads + sumsq
            # emitted during this chunk's FFN to fill PE/A/Pool gaps)
            xk0, vk0 = emit_loads(0)
            rb0 = emit_ss_ln_exp(xk0, 1.0 / D_MODEL, eps_sb, "r1", True)
            carry = (xk0, vk0, emit_xn(xk0, emit_rmm(rb0)))
            for c in range(NCH):
                col0 = c * C
                xk, vk, xn = carry

                # -- B/C projection emitted first: its scalar-engine norm
                # chain (square/ln/relu/exp) then overlaps the conv/dt/xssm
                # matmuls below
                ps_bc = psA.tile([128, C], F32, name="psA", tag="psA")
                for k in range(4):
                    mm(out=ps_bc, lhsT=wbc_sb[k], rhs=xn[k],
                       start=(k == 0), stop=(k == 3))
                sq_bc = pbc.tile([128, C], BF16, name="sqbc", tag="sqbc")
                nc.scalar.activation(out=sq_bc, in_=ps_bc, func=AF.Square)
                bm_s = pbc.tile([128, C], BF16, name="bms", tag="bms")
                nc.vector.tensor_copy(out=bm_s, in_=ps_bc)

                # -- conv input projection, u half only (gates done later to
                # minimize act-table switches)
                cc_t = []
                for m in range(2):
                    ps = psA.tile([128, C], F32, name="psA", tag="psA")
                    for k in range(4):
                        mm(out=ps, lhsT=wconv_sb[k][:, m * 128:(m + 1) * 128],
                           rhs=xn[k], start=(k == 0), stop=(k == 3))
                    ue = pconv.tile([128, C + 2], F32, name="uext", tag="uext")
                    nc.vector.tensor_copy(out=ue[:, 2:C + 2], in_=ps)
                    nc.vector.tensor_copy(out=ue[:, 0:2], in_=u_halo[m])
                    nc.vector.tensor_copy(out=u_halo[m], in_=ue[:, C:C + 2])
                    cc = pconv.tile([128, C], F32, name="cc", tag="cc")
                    nc.vector.tensor_scalar(
                        out=cc, in0=ue[:, 0:C], scalar1=convw[m][:, 0:1],
                        scalar2=convb[m], op0=OP.mult, op1=OP.add,
                    )
                    for kk in (1, 2):
                        nc.vector.scalar_tensor_tensor(
                            out=cc, in0=ue[:, kk:C + kk],
                            scalar=convw[m][:, kk:kk + 1], in1=cc,
                            op0=OP.mult, op1=OP.add,
                        )
                    cc_t.append(cc)

                # -- B/C row-norm reduce; emitted here so the scalar-engine
                # rn chain runs early (and shares the ln/exp table load with
                # the dt chain below)
                ps_n2 = psA.tile([128, C], F32, name="psA", tag="psA")
                mm(out=ps_n2[0:2, :], lhsT=mask2, rhs=sq_bc, start=True, stop=True)
                # min(1, 1/sqrt(z)) = exp(-0.5*relu(ln z)) — all scalar engine
                rn = pbc.tile([2, C], BF16, name="rn", tag="rn")
                rnf = pbc.tile([2, C], F32, name="rnf", tag="rnf")
                nc.scalar.activation(
                    out=rnf, in_=ps_n2[0:2, :], func=AF.Ln, bias=eps_sb[0:2]
                )
                nc.scalar.activation(out=rnf, in_=rnf, func=AF.Relu)
                nc.scalar.activation(out=rn, in_=rnf, func=AF.Exp, scale=-0.5)

                # -- dt chain (softplus = ln(1+exp(x)); stays in ln/exp table)
                dtv, decv = [], []
                for m in range(2):
                    ps = psA.tile([128, C], F32, name="psA", tag="psA")
                    for k in range(4):
                        mm(out=ps, lhsT=wdt_sb[k][:, m * 128:(m + 1) * 128],
                           rhs=xn[k], start=(k == 0), stop=(k == 3))
                    e_t = pssm.tile([128, C], F32, name="dte", tag="dte")
                    nc.scalar.activation(out=e_t, in_=ps, func=AF.Exp, bias=dtb[m])
                    sp_t = pssm.tile([128, C], F32, name="dtsp", tag="dtsp")
                    nc.scalar.activation(out=sp_t, in_=e_t, func=AF.Ln, bias=one_sb)
                    dt_f = pssm.tile([128, C], F32, name="dtf", tag="dtf")
                    nc.vector.tensor_scalar(
                        out=dt_f, in0=sp_t, scalar1=1e-4, scalar2=0.1,
                        op0=OP.max, op1=OP.min,
                    )
                    dtv.append(dt_f)
                    dec = pssm.tile([128, C], F32, name="dec", tag="dec")
                    nc.scalar.activation(out=dec, in_=dt_f, func=AF.Exp, scale=avec[m])
                    decv.append(dec)

                # -- x_ssm; consume the PSUM tiles immediately:
                # t1 = dt*x_ssm (scan input base), Dx = D*x_ssm (skip term)
                t1v, dxv = [], []
                for m in range(2):
                    ps = psA.tile([128, C], F32, name="psA", tag="psA")
                    for k in range(4):
                        mm(out=ps, lhsT=wxp_sb[k][:, m * 128:(m + 1) * 128],
                           rhs=xn[k], start=(k == 0), stop=(k == 3))
                    t1 = pssm.tile([128, C], F32, name="t1", tag="t1")
                    nc.vector.tensor_mul(out=t1, in0=dtv[m], in1=ps)
                    t1v.append(t1)
                    dx = pssm.tile([128, C], F32, name="dx", tag="dx")
                    nc.vector.tensor_scalar(
                        out=dx, in0=ps, scalar1=dvec[m], scalar2=None, op0=OP.mult
                    )
                    dxv.append(dx)

                # -- conv gate (silu) + gated conv output
                conv_out = []
                for m in range(2):
                    psg = psA.tile([128, C], F32, name="psA", tag="psA")
                    for k in range(4):
                        mm(out=psg, lhsT=wconv_sb[k][:, (m + 2) * 128:(m + 3) * 128],
                           rhs=xn[k], start=(k == 0), stop=(k == 3))
                    gs = pconv.tile([128, C], BF16, name="gs", tag="gs")
                    nc.scalar.activation(out=gs, in_=psg, func=AF.Silu)
                    co = pconv.tile([128, C], BF16, name="co", tag="co")
                    nc.gpsimd.tensor_mul(out=co, in0=cc_t[m], in1=gs)
                    conv_out.append(co)

                # -- broadcast rn to 128 partitions, normalize B/C
                ps_rn = psA.tile([128, C], F32, name="psA", tag="psA")
                mm(out=ps_rn, lhsT=selmix, rhs=rn, start=True, stop=True)
                bcn_b = pbc.tile([64, C], BF16, name="bcnb", tag="bcnb")
                nc.vector.scalar_tensor_tensor(
                    out=bcn_b, in0=bm_s[0:64, :], scalar=1.0, in1=ps_rn[0:64, :],
                    op0=OP.bypass, op1=OP.mult,
                )
                bcn_c = pbc.tile([64, C], BF16, name="bcnc", tag="bcnc")
                nc.vector.scalar_tensor_tensor(
                    out=bcn_c, in0=bm_s[64:128, :], scalar=1.0, in1=ps_rn[64:128, :],
                    op0=OP.bypass, op1=OP.mult,
                )
                ps_b128 = psA.tile([128, C], F32, name="psA", tag="psA")
                mm(out=ps_b128, lhsT=dup64, rhs=bcn_b, start=True, stop=True)
                ps_c128 = psA.tile([128, C], F32, name="psA", tag="psA")
                mm(out=ps_c128, lhsT=dup64, rhs=bcn_c, start=True, stop=True)

                # -- scan + y
                yt_t = []
                for m in range(2):
                    inp = pssm.tile([128, C], F32, name="inp", tag="inp")
                    nc.vector.tensor_mul(out=inp, in0=t1v[m], in1=ps_b128)
                    hs = pssm.tile([128, C], F32, name="hs", tag="hs")
                    nc.vector.tensor_tensor_scan(
                        out=hs, data0=decv[m], data1=inp, initial=h_st[m],
                        op0=OP.mult, op1=OP.add,
                    )
                    nc.vector.tensor_copy(out=h_st[m], in_=hs[:, C - 1:C])
                    hc = pssm.tile([128, C], F32, name="hc", tag="hc")
                    nc.vector.tensor_mul(out=hc, in0=hs, in1=ps_c128)
                    yt = pssm.tile([128, C], BF16, name="yt", tag="yt")
                    nc.gpsimd.tensor_add(out=yt, in0=dxv[m], in1=hc)
                    yt_t.append(yt)

                # -- mixer + velocity + residual
                mix_rhs = [conv_out[0], conv_out[1], yt_t[0], yt_t[1]]
                x2 = []
                for m in range(4):
                    ps = psA.tile([128, C], F32, name="psA", tag="psA")
                    for k in range(4):
                        mm(out=ps, lhsT=wcat_sb[k][:, m * 128:(m + 1) * 128],
                           rhs=mix_rhs[k], start=(k == 0), stop=(k == 3))
                    vn = pvn.tile([128, C], F32, name="vn", tag="vn")
                    nc.vector.scalar_tensor_tensor(
                        out=vn, in0=vk[m], scalar=beta, in1=ps,
                        op0=OP.mult, op1=OP.add,
                    )
                    nc.sync.dma_start(
                        out=vo_d[m * 128:(m + 1) * 128, col0:col0 + C], in_=vn
                    )
                    x2t = px2.tile([128, C], F32, name="x2", tag="x2")
                    nc.gpsimd.tensor_add(out=x2t, in0=xk[m], in1=vn)
                    x2.append(x2t)

                # -- FFN norm (scale folded with SA for fp8)
                rb2 = emit_ss_ln_exp(
                    x2, 1.0 / (D_MODEL * SA * SA), eps2_sb, "r2", False
                )
                ps_r2 = emit_rmm(rb2)
                x2n = []
                for j in range(2):
                    t = px2n.tile([128, 2, C], FP8, name="x2n", tag="x2n")
                    for i in range(2):
                        nc.vector.tensor_mul(
                            out=t[:, i, :], in0=x2[2 * j + i], in1=ps_r2
                        )
                    x2n.append(t)

                # -- pipelined head of next chunk: loads + sumsq + ln/exp land
                # here so they overlap this chunk's FFN
                if c + 1 < NCH:
                    xk1, vk1 = emit_loads(c + 1)
                    rb1 = emit_ss_ln_exp(xk1, 1.0 / D_MODEL, eps_sb, "r1", True)

                # -- FFN: w1/w3 (fp8 DR) -> silu/gate -> h (fp8) -> w2 (fp8 DR)
                hdr = []
                for p in range(8):
                    hdr.append(phdr.tile([128, 2, C], FP8, name="hdr", tag=f"hdr{p}"))
                cs1 = 1.0 / (SA * S1)
                for pp in range(8):
                    # a-halves for kf=2pp, 2pp+1 into one 2-bank tile so a
                    # single silu covers both
                    pa = psB.tile([128, 2 * C], F32, name="psB", tag="psB")
                    for half in range(2):
                        kf = 2 * pp + half
                        for j in range(2):
                            mm(out=pa[:, half * C:(half + 1) * C],
                               lhsT=w1_sb[j][:, :, kf * 128:(kf + 1) * 128],
                               rhs=x2n[j], start=(j == 0), stop=(j == 1), pm=DR)
                    sa_t = psil.tile([128, 2 * C], BF16, name="sat", tag="sat")
                    nc.scalar.activation(out=sa_t, in_=pa, func=AF.Silu, scale=cs1)
                    for half in range(2):
                        kf = 2 * pp + half
                        ps_b = psA.tile([128, C], F32, name="psA", tag="psA")
                        for j in range(2):
                            mm(out=ps_b,
                               lhsT=w3_sb[j][:, :, kf * 128:(kf + 1) * 128],
                               rhs=x2n[j], start=(j == 0), stop=(j == 1), pm=DR)
                        # ps_b already carries SH scaling (via S3 fold on host)
                        nc.vector.tensor_mul(
                            out=hdr[pp][:, half, :], in0=ps_b,
                            in1=sa_t[:, half * C:(half + 1) * C],
                        )
                # r broadcast + xn for next chunk: emitted after the FFN pairs
                # so the PE doesn't stall on the ln/exp chain, and xn lands on
                # DVE before this chunk's xo ops
                if c + 1 < NCH:
                    carry = (xk1, vk1, emit_xn(xk1, emit_rmm(rb1)))

                cf = 1.0 / (SH * S2)
                for m in range(4):
                    pf = psF.tile([128, C], F32, name="psF", tag="psF")
                    for j in range(8):
                        mm(out=pf, lhsT=w2_sb[j][:, :, m * 128:(m + 1) * 128],
                           rhs=hdr[j], start=(j == 0), stop=(j == 7), pm=DR)
                    xo = pxo.tile([128, C], F32, name="xo", tag="xo")
                    nc.vector.scalar_tensor_tensor(
                        out=xo, in0=pf, scalar=cf, in1=x2[m],
                        op0=OP.mult, op1=OP.add,
                    )
                    nc.sync.dma_start(
                        out=xo_d[m * 128:(m + 1) * 128, col0:col0 + C], in_=xo
                    )

    if split:
        split_waits(nc)
    return nc


# ---------------------------------------------------------------- host glue
def prep_weights(inputs):
    f = lambda a: np.asarray(a, dtype=np.float32)
    pre_w = f(inputs["pre_norm_w"])[:, None]
    ffn_w = f(inputs["ffn_norm_w"])[:, None]
    A = -np.exp(f(inputs["A_log"]).reshape(-1))
    beta = float(1.0 / (1.0 + np.exp(-f(inputs["log_beta"]))))

    mask2 = np.zeros((128, 2), np.float32)
    mask2[0:64, 0] = 1.0
    mask2[64:128, 1] = 1.0
    selmix = np.zeros((2, 128), np.float32)
    selmix[0, 0:64] = 1.0
    selmix[1, 64:128] = 1.0
    dup64 = np.zeros((64, 128), np.float32)
    for p in range(128):
        dup64[p % 64, p] = 1.0

    wop = f(inputs["out_proj_w"])
    wcat = np.concatenate(
        [wop[0:D_CONV, :], f(inputs["ssm_out_w"]) @ wop[D_CONV:, :]], axis=0
    )

    def dr_pack(w, njt):
        # w: [K, N] fp8-scaled; -> [njt*128, 2, N] with [j*128+p, i, n] =
        # w[(2j+i)*128 + p, n]
        K, N = w.shape
        out = np.zeros((njt * 128, 2, N), w.dtype)
        for j in range(njt):
            for i in range(2):
                out[j * 128:(j + 1) * 128, i, :] = w[(2 * j + i) * 128:(2 * j + i + 1) * 128, :]
        return out

    bf = ml_dtypes.bfloat16
    f8 = ml_dtypes.float8_e4m3fn
    w1q = ((ffn_w * f(inputs["w1"])) * S1).astype(f8)
    w3q = ((ffn_w * f(inputs["w3"])) * S3).astype(f8)
    w2q = (f(inputs["w2"]) * S2).astype(f8)

    w = {
        "w_conv": np.ascontiguousarray((pre_w * f(inputs["conv_in_w"])).astype(bf)),
        "w_xproj": np.ascontiguousarray((pre_w * f(inputs["x_proj_w"])).astype(bf)),
        "w_dt": np.ascontiguousarray((pre_w * f(inputs["dt_w"])).astype(bf)),
        "w_bc": np.ascontiguousarray(
            (pre_w * np.concatenate([f(inputs["B_w"]), f(inputs["C_w"])], axis=1)).astype(bf)
        ),
        "w_cat": np.ascontiguousarray(wcat.astype(bf)),
        "w1dr": np.ascontiguousarray(dr_pack(w1q, 2)),
        "w3dr": np.ascontiguousarray(dr_pack(w3q, 2)),
        "w2dr": np.ascontiguousarray(dr_pack(w2q, 8)),
        "a_vec": A[:, None].copy(),
        "dtb_vec": f(inputs["dt_b"])[:, None].copy(),
        "d_vec": f(inputs["D"])[:, None].copy(),
        "convb_vec": f(inputs["conv_dw_b"])[:, None].copy(),
        "convw": np.ascontiguousarray(f(inputs["conv_dw_w"])),
        "mask2": mask2.astype(bf),
        "selmix": selmix.astype(bf),
        "dup64": dup64.astype(bf),
        "ones128": np.ones((128, 1), bf),
        "ones1": np.ones((1, 128), bf),
    }
    return w, beta


CHUNK = 512

_PROG_CACHE = {}


def kernel(**inputs):
    """Batch-parallel over 8 cores; transposed I/O handled on host."""
    w, beta = prep_weights(inputs)
    x = np.asarray(inputs["x"], np.float32)
    v = np.asarray(inputs["velocity"], np.float32)
    n_cores, L, _ = x.shape
    key = (L, CHUNK, beta)
    if key not in _PROG_CACHE:
        _PROG_CACHE[key] = build_program(L, CHUNK, beta)
    nc = _PROG_CACHE[key]
    in_maps = []
    for b in range(n_cores):
        m = dict(w)
        m["x"] = np.ascontiguousarray(x[b].T)
        m["v"] = np.ascontiguousarray(v[b].T)
        in_maps.append(m)
    res = run_bass_kernel_spmd(nc, in_maps, core_ids=list(range(n_cores)))
    x_out = np.stack([res.results[b]["x_out"].T for b in range(n_cores)])
    v_out = np.stack([res.results[b]["v_out"].T for b in range(n_cores)])
    return (x_out, v_out)
